# revision 1
# baseline (speedup 1.0000x reference)
"""Trainium2 Bass kernel for nn_CTCPerSpeakerExtractorConcatNNG.

Sharding: 8 cores = (batch b, speaker k) pairs; each core runs the full
T=1536 stream for its pair. No collectives; host scatters/gathers.

Per-core dataflow (natural layout [T-tiles x 128 part, D free], bf16 acts):
  X = xmT.T @ Win + bin               (xmT pre-transposed on host)
  LN_kv(X) -> transpose -> KVT -> KT (transposed), V (halo-tiled natural)
  Xk = X * sigmoid(6(A-.5));  LN_q -> transpose -> QT (transposed)
  banded attention (BAND=24) with 128-query tiles x 176-key windows
  y2 = Xk + attn@Wo ; LN_f -> transpose -> FFN (gelu) ; y3 = y2 + h2 + b2k
  out = LN_s(y3) normalized only; host applies ln_s gain/bias.
LN gains/biases for kv/q/f are folded into the following matmul on host.
"""
import sys

for _p in ("/opt/trn_rl_repo", "/root/.axon_site/_ro/trn_rl_repo"):
    if _p not in sys.path:
        sys.path.append(_p)

from contextlib import ExitStack

import numpy as np
import ml_dtypes

import concourse.bass as bass
import concourse.bacc as bacc
import concourse.tile as tile
from concourse import mybir
from concourse.bass_utils import run_bass_kernel_spmd
from concourse.masks import make_identity

BF = mybir.dt.bfloat16
F32 = mybir.dt.float32
AF = mybir.ActivationFunctionType
OP = mybir.AluOpType

B, T, D, KSP, H, BAND = 4, 1536, 512, 2, 8, 24
DH = D // H          # 64
P = 128
NT = T // P          # 12
WIN = P + 2 * BAND   # 176
NC_D = D // P        # 4 chunks of contraction dim
DFF = 4 * D          # 2048
NDH = DFF // P       # 16
EPS = 1e-5

# V halo-tile starts (each tile = up to 128 rows starting at s)
_VSTARTS = sorted({0, 128, 1360, 1488} | {128 * m - 24 for m in range(1, 12)})
_VIDX = {s: j for j, s in enumerate(_VSTARTS)}
NV = len(_VSTARTS)   # 15


def _bcast_ap(dram_ap, parts=128):
    """[N] dram vector -> [parts, N] broadcast AP (partition step 0)."""
    return bass.AP(
        tensor=dram_ap.tensor,
        offset=dram_ap.offset,
        ap=[[0, parts]] + list(dram_ap.ap),
    )


def _ln_stats(nc, pool, in_ap, eps_t):
    """Return (rstd[128,1] f32, negm_rstd[128,1] f32) for LN over free dim."""
    st = pool.tile([P, 6], F32)
    nc.vector.bn_stats(out=st, in_=in_ap)
    mv = pool.tile([P, 2], F32)
    nc.vector.bn_aggr(out=mv, in_=st)
    sd = pool.tile([P, 1], F32)
    nc.scalar.activation(out=sd, in_=mv[:, 1:2], func=AF.Sqrt, bias=eps_t)
    rstd = pool.tile([P, 1], F32)
    nc.vector.reciprocal(out=rstd, in_=sd)
    return mv[:, 0:1], rstd


def build_program(add_bo: bool, stop_stage: int = 99, add_bin: bool = False,
                  add_bv: bool = False) -> bass.Bass:
    nc = bacc.Bacc()

    # ---- DRAM I/O ----
    xmT = nc.dram_tensor("xmT", [D, T], BF, kind="ExternalInput")
    Wd = {}
    for nm, (di, do) in [("Win", (D, D)), ("Wq", (D, D)), ("Wk", (D, D)),
                         ("Wv", (D, D)), ("Wo", (D, D)), ("W1", (D, DFF)),
                         ("W2", (DFF, D))]:
        Wd[nm] = nc.dram_tensor(nm, [di, do], BF, kind="ExternalInput")
    smalls_d = nc.dram_tensor("smalls", [P, 36], F32, kind="ExternalInput")
    rows_d = nc.dram_tensor("rows", [4, D], F32, kind="ExternalInput")
    out_d = nc.dram_tensor("out", [T, D], F32, kind="ExternalOutput")
    out_t = out_d.rearrange("(n p) d -> n p d", p=P)

    with tile.TileContext(nc) as tc, ExitStack() as ctx:
        def _body():
            consts = ctx.enter_context(tc.tile_pool(name="consts", bufs=1))
            wpool = ctx.enter_context(tc.tile_pool(name="wpool", bufs=1))
            acts = ctx.enter_context(tc.tile_pool(name="acts", bufs=1))
            ln_nat_p = ctx.enter_context(tc.tile_pool(name="ln_nat_p", bufs=2))
            tT_p = ctx.enter_context(tc.tile_pool(name="tT_p", bufs=1))
            kqh = ctx.enter_context(tc.tile_pool(name="kqh", bufs=2))
            small = ctx.enter_context(tc.tile_pool(name="small", bufs=6))
            sm2 = ctx.enter_context(tc.tile_pool(name="sm2", bufs=2))
            outp = ctx.enter_context(tc.tile_pool(name="outp", bufs=2))
            psA = ctx.enter_context(tc.tile_pool(name="psA", bufs=2, space="PSUM"))
            psT = ctx.enter_context(tc.tile_pool(name="psT", bufs=1, space="PSUM"))
            psB = ctx.enter_context(tc.tile_pool(name="psB", bufs=2, space="PSUM"))
            psC = ctx.enter_context(tc.tile_pool(name="psC", bufs=2, space="PSUM"))
            psD = ctx.enter_context(tc.tile_pool(name="psD", bufs=1, space="PSUM"))

            def dbg_out(get_tile_view, grouped=False):
                # copy 12 [128,512] views (cast to f32) to out and stop
                for _mt in range(NT):
                    o = outp.tile([P, D], F32, tag="o_sb")
                    ov = o.rearrange("p (c q) -> p c q", c=NC_D) if grouped else o
                    nc.vector.tensor_copy(out=ov, in_=get_tile_view(_mt))
                    nc.sync.dma_start(out=out_t[_mt], in_=o)

            # ---- constants ----
            ident = consts.tile([P, P], BF)
            make_identity(nc, ident)
            eps_t = consts.tile([P, 1], F32, tag="eps_t")
            nc.vector.memset(eps_t, EPS)
            neg3_t = consts.tile([P, 1], F32, tag="neg3_t")
            nc.vector.memset(neg3_t, -3.0)
            masks = {}
            NEG = -1e30
            for off, nm in ((0, "mid"), (-BAND, "first"), (BAND, "last")):
                mk = consts.tile([P, WIN], BF, tag=f"mask_{nm}")
                nc.gpsimd.memset(mk, 0.0)
                # keep iff (kj - qi - off) >= 0 else -inf
                nc.gpsimd.affine_select(
                    out=mk, in_=mk, compare_op=OP.is_ge, fill=NEG,
                    base=-off, pattern=[[1, WIN]], channel_multiplier=-1)
                # keep iff (qi + off + 2*BAND - kj) >= 0 else -inf
                nc.gpsimd.affine_select(
                    out=mk, in_=mk, compare_op=OP.is_ge, fill=NEG,
                    base=off + 2 * BAND, pattern=[[-1, WIN]], channel_multiplier=1)
                masks[nm] = mk

            def mask_for(mt):
                return masks["first" if mt == 0 else ("last" if mt == NT - 1 else "mid")]

            def ws_of(mt):
                return min(max(mt * P - BAND, 0), T - WIN)

            xmT_s = ln_nat_p.tile([P, NC_D, T], BF, tag="ln_nat")
            nc.sync.dma_start(out=xmT_s, in_=xmT.rearrange("(c p) t -> p c t", p=P))

            # ---- weights/biases to SBUF (order: earliest-needed first) ----
            Ws = {}
            for nm in ("Win", "Wk", "Wv", "Wq", "Wo", "W1"):
                di, do = Wd[nm].shape
                t = wpool.tile([P, di // P, do], BF, tag=nm)
                nc.sync.dma_start(out=t, in_=Wd[nm].rearrange("(c p) o -> p c o", p=P))
                Ws[nm] = t
            W2s = wpool.tile([P, NDH, D], BF, tag="W2")
            nc.sync.dma_start(out=W2s, in_=Wd["W2"].rearrange("(c p) o -> p c o", p=P))

            # packed small tensors: cols [0:12]=A_k tiled, [12:16]=bq4,
            # [16:20]=bk4, [20:36]=b1_16 (host-packed, one contiguous DMA)
            smalls = consts.tile([P, 36], F32, tag="smalls")
            nc.sync.dma_start(out=smalls, in_=smalls_d[:, :])
            Ak_s = smalls[:, 0:NT]
            bq4 = smalls[:, 12:12 + NC_D]
            bk4 = smalls[:, 16:16 + NC_D]
            b1_16 = smalls[:, 20:20 + NDH]
            # bias row broadcasts (partition-step-0 DMA reads, contiguous source)
            bin_b = consts.tile([P, D], F32, tag="bin_b")
            nc.sync.dma_start(out=bin_b, in_=_bcast_ap(rows_d[0, :]))
            bv_b = consts.tile([P, D], F32, tag="bv_b")
            nc.sync.dma_start(out=bv_b, in_=_bcast_ap(rows_d[1, :]))
            b2k_b = consts.tile([P, D], F32, tag="b2k_b")
            nc.sync.dma_start(out=b2k_b, in_=_bcast_ap(rows_d[2, :]))
            if add_bo:
                ones_r = consts.tile([1, P], BF, tag="ones_r")
                nc.vector.memset(ones_r, 1.0)
                bo_rf = consts.tile([1, D], F32, tag="bo_rf")
                nc.sync.dma_start(out=bo_rf, in_=rows_d[3:4, :])
                bo_rb = consts.tile([1, D], BF, tag="bo_rb")
                nc.vector.tensor_copy(out=bo_rb, in_=bo_rf)

            What = consts.tile([P, NT], F32, tag="What")
            nc.scalar.activation(out=What, in_=Ak_s, func=AF.Sigmoid, scale=6.0, bias=neg3_t)

            # ---- X = xmT.T @ Win + bin ; LN_kv ; Xk (X stays in PSUM only) ----
            lnkv = ln_nat_p.tile([P, NT, D], BF, tag="ln_nat")
            Xk = acts.tile([P, NT, D], BF, tag="Xk")
            for mt in range(NT):
                ps = psA.tile([P, D], F32, tag="psA")
                for c in range(NC_D):
                    nc.tensor.matmul(
                        ps, lhsT=xmT_s[:, c, mt * P:(mt + 1) * P], rhs=Ws["Win"][:, c, :],
                        start=(c == 0), stop=(c == NC_D - 1))
                if add_bin:
                    psb = sm2.tile([P, D], F32, tag="Xpsb")
                    nc.vector.tensor_tensor(out=psb, in0=ps, in1=bin_b, op=OP.add)
                else:
                    psb = ps
                mean, rstd = _ln_stats(nc, small, psb, eps_t)
                negm = small.tile([P, 1], F32, tag="negm")
                nc.vector.tensor_scalar(out=negm, in0=mean, scalar1=rstd,
                                        scalar2=-1.0, op0=OP.mult, op1=OP.mult)
                nc.scalar.activation(out=lnkv[:, mt, :], in_=psb, func=AF.Identity,
                                     scale=rstd, bias=negm)
                nc.scalar.activation(out=Xk[:, mt, :], in_=psb, func=AF.Copy,
                                     scale=What[:, mt:mt + 1])

            if stop_stage == 1:
                dbg_out(lambda m: lnkv[:, m, :])
                return

            # ---- transpose LN_kv -> KVT [128, 4, T] ----
            def transpose_nat_to_T(src, dst):
                for mt in range(NT):
                    pt = psT.tile([P, NC_D * P], BF, tag="psT")
                    for c in range(NC_D):
                        nc.tensor.transpose(
                            pt[:, c * P:(c + 1) * P], src[:, mt, c * P:(c + 1) * P], ident)
                    nc.vector.tensor_copy(
                        out=dst[:, :, mt * P:(mt + 1) * P],
                        in_=pt.rearrange("p (c q) -> p c q", c=NC_D))

            KVT = tT_p.tile([P, NC_D, T], BF, tag="tT")
            transpose_nat_to_T(lnkv, KVT)

            if stop_stage == 2:
                dbg_out(lambda m: KVT[:, :, m * P:(m + 1) * P], grouped=True)
                return

            # ---- KT (transposed) and V (halo natural) ----
            KT = kqh.tile([P, NC_D, T], BF, tag="kqh")
            for co in range(NC_D):
                for tch in range(3):
                    ps = psA.tile([P, D], F32, tag="psA")
                    for c in range(NC_D):
                        nc.tensor.matmul(
                            ps, lhsT=Ws["Wk"][:, c, co * P:(co + 1) * P],
                            rhs=KVT[:, c, tch * D:(tch + 1) * D],
                            start=(c == 0), stop=(c == NC_D - 1))
                    nc.scalar.activation(
                        out=KT[:, co, tch * D:(tch + 1) * D], in_=ps,
                        func=AF.Identity, bias=bk4[:, co:co + 1])

            Vh = acts.tile([P, NV, D], BF, tag="Vh")
            for j, s in enumerate(_VSTARTS):
                w = min(P, T - s)
                ps = psA.tile([P, D], F32, tag="psA")
                for c in range(NC_D):
                    nc.tensor.matmul(
                        ps[0:w, :], lhsT=KVT[:, c, s:s + w], rhs=Ws["Wv"][:, c, :],
                        start=(c == 0), stop=(c == NC_D - 1))
                if add_bv:
                    nc.vector.tensor_tensor(
                        out=Vh[0:w, j, :], in0=ps[0:w, :], in1=bv_b[0:w, :], op=OP.add)
                else:
                    nc.scalar.copy(out=Vh[0:w, j, :], in_=ps[0:w, :])

            if stop_stage == 3:
                dbg_out(lambda m: Vh[:, m, :])
                return

            # ---- LN_q on Xk ; transpose ; QT ----
            lnq = ln_nat_p.tile([P, NT, D], BF, tag="ln_nat")
            for mt in range(NT):
                mean, rstd = _ln_stats(nc, small, Xk[:, mt, :], eps_t)
                negm = small.tile([P, 1], F32, tag="negm")
                nc.vector.tensor_scalar(out=negm, in0=mean, scalar1=rstd,
                                        scalar2=-1.0, op0=OP.mult, op1=OP.mult)
                nc.scalar.activation(out=lnq[:, mt, :], in_=Xk[:, mt, :],
                                     func=AF.Identity, scale=rstd, bias=negm)
            LNQT = tT_p.tile([P, NC_D, T], BF, tag="tT")
            transpose_nat_to_T(lnq, LNQT)

            QT = kqh.tile([P, NC_D, T], BF, tag="kqh")
            for co in range(NC_D):
                for tch in range(3):
                    ps = psA.tile([P, D], F32, tag="psA")
                    for c in range(NC_D):
                        nc.tensor.matmul(
                            ps, lhsT=Ws["Wq"][:, c, co * P:(co + 1) * P],
                            rhs=LNQT[:, c, tch * D:(tch + 1) * D],
                            start=(c == 0), stop=(c == NC_D - 1))
                    nc.scalar.activation(
                        out=QT[:, co, tch * D:(tch + 1) * D], in_=ps,
                        func=AF.Identity, bias=bq4[:, co:co + 1])

            if stop_stage == 4:
                dbg_out(lambda m: QT[:, :, m * P:(m + 1) * P], grouped=True)
                return

            # ---- attention ----
            YT = acts.tile([P, NC_D, T], BF, tag="YT")
            inv_sqrt_dh = 1.0 / float(np.sqrt(DH))
            for mt in range(NT):
                ws = ws_of(mt)
                mk = mask_for(mt)
                den = small.tile([P, H], F32, tag="den")
                pm_a = sm2.tile([P, H, WIN], BF, tag="pm_a")
                for h in range(H):
                    hp, hc = 64 * (h % 2), h // 2
                    ps = psB.tile([P, WIN], F32, tag="psB")
                    nc.tensor.matmul(
                        ps, lhsT=QT[hp:hp + 64, hc, mt * P:(mt + 1) * P],
                        rhs=KT[hp:hp + 64, hc, ws:ws + WIN], start=True, stop=False)
                    # accumulate additive band mask: ps += ident.T @ mk = mk
                    nc.tensor.matmul(ps, lhsT=ident, rhs=mk, start=False, stop=True)
                    if stop_stage == 41:
                        nc.scalar.activation(out=pm_a[:, h, :], in_=ps,
                                             func=AF.Exp, scale=inv_sqrt_dh)
                        continue
                    # exp with row-sum accumulation -> masked softmax numerator+denominator
                    nc.scalar.activation(out=pm_a[:, h, :], in_=ps, func=AF.Exp,
                                         scale=inv_sqrt_dh, accum_out=den[:, h:h + 1])
                if stop_stage in (41, 42):
                    dbg_pm = pm_a
                    continue
                r8 = small.tile([P, H], F32, tag="r8")
                nc.vector.reciprocal(out=r8, in_=den)
                psy = psD.tile([P, D], F32, tag="psD")
                if mt == 0:
                    j1, j2 = _VIDX[0], _VIDX[128]
                elif mt == NT - 1:
                    j1, j2 = _VIDX[1360], _VIDX[1488]
                else:
                    j1, j2 = _VIDX[128 * mt - 24], _VIDX[128 * mt + 104]
                for h in range(H):
                    hp, hc = 64 * (h % 2), h // 2
                    pms = sm2.tile([P, WIN], BF, tag="pms")
                    nc.vector.tensor_scalar_mul(
                        out=pms, in0=pm_a[:, h, :], scalar1=r8[:, h:h + 1])
                    if stop_stage == 43:
                        dbg_pm = pms
                        continue
                    ptp = psC.tile([P, 2 * P], BF, tag="psC")
                    nc.tensor.transpose(ptp[:, 0:P], pms[:, 0:P], ident)
                    nc.tensor.transpose(ptp[0:48, P:2 * P], pms[:, P:WIN], ident)
                    pts = sm2.tile([P, 2 * P], BF, tag="pts")
                    nc.vector.tensor_copy(out=pts[:, 0:P], in_=ptp[:, 0:P])
                    nc.scalar.copy(out=pts[0:48, P:2 * P], in_=ptp[0:48, P:2 * P])
                    if stop_stage == 44:
                        dbg_pm = pts
                        continue
                    nc.tensor.matmul(
                        psy[hp:hp + 64, hc * P:(hc + 1) * P],
                        lhsT=Vh[:, j1, h * DH:(h + 1) * DH], rhs=pts[:, 0:P],
                        start=True, stop=False)
                    nc.tensor.matmul(
                        psy[hp:hp + 64, hc * P:(hc + 1) * P],
                        lhsT=Vh[0:48, j2, h * DH:(h + 1) * DH],
                        rhs=pts[0:48, P:2 * P],
                        start=False, stop=True)
                if stop_stage in (43, 44):
                    continue
                nc.vector.tensor_copy(
                    out=YT[:, :, mt * P:(mt + 1) * P],
                    in_=psy.rearrange("p (c q) -> p c q", c=NC_D))

            if stop_stage in (41, 42):
                dv = dbg_pm.rearrange("p h w -> p (h w)")
                dbg_out(lambda m: dv[:, 0:D])
                return
            if stop_stage in (43, 44):
                for _mt in range(NT):
                    o = outp.tile([P, D], F32, tag="o_sb")
                    nc.vector.tensor_copy(out=o[:, 0:dbg_pm.shape[-1]], in_=dbg_pm)
                    nc.sync.dma_start(out=out_t[_mt], in_=o)
                return
            if stop_stage == 5:
                dbg_out(lambda m: YT[:, :, m * P:(m + 1) * P], grouped=True)
                return

            # ---- attn out + residual: y2 = Xk + YT.T @ Wo (+ bo) ----
            y2 = acts.tile([P, NT, D], BF, tag="y2")
            lnf = ln_nat_p.tile([P, NT, D], BF, tag="ln_nat")
            for mt in range(NT):
                ps = psA.tile([P, D], F32, tag="psA")
                for c in range(NC_D):
                    nc.tensor.matmul(
                        ps, lhsT=YT[:, c, mt * P:(mt + 1) * P], rhs=Ws["Wo"][:, c, :],
                        start=(c == 0), stop=(c == NC_D - 1 and not add_bo))
                if add_bo:
                    nc.tensor.matmul(ps, lhsT=ones_r[:, 0:P], rhs=bo_rb,
                                     start=False, stop=True)
                nc.vector.tensor_tensor(
                    out=y2[:, mt, :], in0=ps, in1=Xk[:, mt, :], op=OP.add)
                mean, rstd = _ln_stats(nc, small, y2[:, mt, :], eps_t)
                nc.vector.tensor_scalar(
                    out=lnf[:, mt, :], in0=y2[:, mt, :], scalar1=mean, scalar2=rstd,
                    op0=OP.subtract, op1=OP.mult)

            if stop_stage == 6:
                dbg_out(lambda m: lnf[:, m, :])
                return

            LNFT = tT_p.tile([P, NC_D, T], BF, tag="tT")
            transpose_nat_to_T(lnf, LNFT)

            # ---- FFN + final residual + LN_s + output ----
            for tch in range(3):
                H1g = kqh.tile([P, NDH, D], BF, tag="kqh")
                for dh in range(NDH):
                    ps = psA.tile([P, D], F32, tag="psA")
                    for c in range(NC_D):
                        nc.tensor.matmul(
                            ps, lhsT=Ws["W1"][:, c, dh * P:(dh + 1) * P],
                            rhs=LNFT[:, c, tch * D:(tch + 1) * D],
                            start=(c == 0), stop=(c == NC_D - 1))
                    nc.scalar.activation(out=H1g[:, dh, :], in_=ps, func=AF.Gelu,
                                         bias=b1_16[:, dh:dh + 1])
                for sub in range(4):
                    mtg = tch * 4 + sub
                    ps = psA.tile([P, D], F32, tag="psA")
                    for dh in range(NDH):
                        nc.tensor.matmul(
                            ps, lhsT=H1g[:, dh, sub * P:(sub + 1) * P],
                            rhs=W2s[:, dh, :], start=(dh == 0), stop=(dh == NDH - 1))
                    y3 = outp.tile([P, D], F32, tag="y3")
                    nc.vector.tensor_tensor(out=y3, in0=ps, in1=y2[:, mtg, :], op=OP.add)
                    y3b = outp.tile([P, D], F32, tag="y3b")
                    nc.gpsimd.tensor_tensor(out=y3b, in0=y3, in1=b2k_b, op=OP.add)
                    mean, rstd = _ln_stats(nc, small, y3b, eps_t)
                    o_sb = outp.tile([P, D], F32, tag="o_sb")
                    nc.vector.tensor_scalar(
                        out=o_sb, in0=y3b, scalar1=mean, scalar2=rstd,
                        op0=OP.subtract, op1=OP.mult)
                    nc.sync.dma_start(out=out_t[mtg], in_=o_sb)

        _body()
    nc.finalize()
    return nc


_PROG_CACHE = {}


def kernel(**inputs) -> np.ndarray:
    f32 = np.float32
    bf = ml_dtypes.bfloat16
    x_m = np.asarray(inputs["x_m"], f32)
    A = np.asarray(inputs["A"], f32)
    g = {k: np.asarray(v, f32) for k, v in inputs.items()}

    # fold LN affine params into following matmuls (exact algebra)
    Wq = g["ln_q_g"][:, None] * g["Wq"]
    bq = g["bq"] + g["ln_q_b"] @ g["Wq"]
    Wk = g["ln_kv_g"][:, None] * g["Wk"]
    bk = g["bk"] + g["ln_kv_b"] @ g["Wk"]
    Wv = g["ln_kv_g"][:, None] * g["Wv"]
    bv = g["bv"] + g["ln_kv_b"] @ g["Wv"]
    W1 = g["ln_f_g"][:, None] * g["W1"]
    b1 = g["b1"] + g["ln_f_b"] @ g["W1"]

    add_bo = bool(np.any(g["bo"] != 0.0))
    add_bin = bool(np.any(g["b_in"] != 0.0))
    add_bv = bool(np.any(bv != 0.0))
    key = (add_bo, add_bin, add_bv)
    if key not in _PROG_CACHE:
        _PROG_CACHE[key] = build_program(add_bo, add_bin=add_bin, add_bv=add_bv)
    nc = _PROG_CACHE[key]

    common = {
        "Win": np.ascontiguousarray(g["W_in"].astype(bf)),
        "Wq": np.ascontiguousarray(Wq.astype(bf)),
        "Wk": np.ascontiguousarray(Wk.astype(bf)),
        "Wv": np.ascontiguousarray(Wv.astype(bf)),
        "Wo": np.ascontiguousarray(g["Wo"].astype(bf)),
        "W1": np.ascontiguousarray(W1.astype(bf)),
        "W2": np.ascontiguousarray(g["W2"].astype(bf)),
    }
    in_maps = []
    for c in range(8):
        b, k = c // 2, c % 2
        im = dict(common)
        im["xmT"] = np.ascontiguousarray(x_m[b].T.astype(bf))
        sm = np.zeros((128, 36), f32)
        sm[:, 0:12] = A[b, :, k].reshape(12, 128).T
        sm[:, 12:16] = bq.reshape(4, 128).T
        sm[:, 16:20] = bk.reshape(4, 128).T
        sm[:, 20:36] = b1.reshape(16, 128).T
        im["smalls"] = sm
        rows = np.stack([g["b_in"], bv, g["b2"] + g["spk_tags"][k], g["bo"]])
        im["rows"] = rows.astype(f32)
        in_maps.append(im)

    res = run_bass_kernel_spmd(nc, in_maps, core_ids=list(range(8)))
    out = np.zeros((B, KSP * T, D), f32)
    gs, bs = g["ln_s_g"], g["ln_s_b"]
    for c in range(8):
        b, k = c // 2, c % 2
        out[b, k * T:(k + 1) * T] = res.results[c]["out"] * gs + bs
    return out

